# revision 11
# baseline (speedup 1.0000x reference)
"""Distributed Trainium2 (Bass/Tile) kernel for the contrastive loss.

Ring-partitioned symmetric-similarity scheme (8 NeuronCores, SPMD):

  Global per l: 4096 rows in 32 chunks of 128.  sim = Z Z^T is symmetric,
  so each unordered chunk pair is computed ONCE: chunk i covers column
  chunks {i..i+15} (ring-forward), and the distance-16 block is computed
  by both endpoints with its exp HALVED (bias ln 1/2).  Core c owns row
  chunks {4c..4c+3}; it therefore only needs Z chunks {4c..4c+19} (a
  20-chunk window, rolled so the window is local chunks 0..19).

  Per core:
    - load raw window rows in natural layout [128, l, 20, 128] (fp32),
    - ssq via fused square+row-sum on DVE; invn = exp(-0.5 ln ssq) (ACT),
    - scale rows by invn -> bf16 zb (GpSimd tensor_scalar),
    - transpose zb via the XBAR DMA-transpose -> xt [d, chunk, row] bf16,
      (prep chain runs in quarters so the first matmul starts early),
    - row-chunk i: 4x 512-col bf16 matmuls -> PSUM [128, 2048];
      ACT exp (scale 1/T) -> E bf16 SBUF + per-row accum (denominator
      row part);  per covered column chunk a 1-col ones-matmul on PE
      computes the column sums (the denominator part owed to OTHER
      row chunks); distance-16 block separately with bias ln(1/2),
    - positive pairs are exactly the distance-16 diagonals:
      pos = zb[:,i,:] . zb[:,i+16,:] row-dots on DVE.
  Outputs per core (one packed DMA): row accums, d16 row sums, pos dots
  and column-sum partials.  The host does the tiny cross-core assembly:
  denominators = row part + mapped column partials - e^5 (self term),
  then loss = sum w (-pos/T + log denom) / (2 sum w).
"""

import numpy as np

TEMP = 0.2
INV_T = 1.0 / TEMP
L, B, K, D = 4, 64, 32, 128
N = B * K            # 2048
M = 2 * N            # 4096 rows per l
NCH = 32             # global 128-row chunks per l
NCORES = 8
RC = 4               # row chunks owned per core
W = 20               # chunk window per core (rc spans + d16 partners)
SPAN = 16            # forward span chunks (excl. the halved d16 block)
QS = 5               # prep quarter size (chunks per quarter)

_built = None


def _build():
    global _built
    if _built is not None:
        return _built
    from contextlib import ExitStack

    import concourse.tile as tile
    from concourse import bacc
    import concourse.mybir as mybir

    f32 = mybir.dt.float32
    bf16 = mybir.dt.bfloat16
    AF = mybir.ActivationFunctionType
    OP = mybir.AluOpType
    AX = mybir.AxisListType

    # Pin every ACT op to the natural_log_exp_and_others table set (covers
    # Copy/Exp/Identity/Ln) so bacc emits exactly one LoadActFuncSet.
    from concourse import hw_specs as _hw
    _tabs = dict(_hw.get_activation_tables("gen3"))
    _pinned = {
        name: (fns if name == "natural_log_exp_and_others" else frozenset())
        for name, fns in _tabs.items()
    }
    _hw.get_activation_tables.cache_clear()
    _orig = _hw.get_activation_tables.__wrapped__

    def _patched(arch):
        if arch == "gen3":
            return _pinned
        return _orig(arch)

    _hw.get_activation_tables = _patched
    import concourse.bacc as _baccmod
    if hasattr(_baccmod, "get_activation_tables"):
        _baccmod.get_activation_tables = _patched

    nc = bacc.Bacc(None, target_bir_lowering=False)
    emb = nc.dram_tensor("emb_nat", [128, L, W, D], f32, kind="ExternalInput")
    # packed: [dsum(16) | d16r(16) | pos(16) | cacc(L*W=80)] = 128 cols
    out = nc.dram_tensor("out_all", [128, 3 * L * RC + L * W], f32,
                         kind="ExternalOutput")

    with ExitStack() as ctx:
        tc = ctx.enter_context(tile.TileContext(nc))
        singles = ctx.enter_context(tc.tile_pool(name="singles", bufs=1))
        natp = ctx.enter_context(tc.tile_pool(name="nat", bufs=2))
        zbp = ctx.enter_context(tc.tile_pool(name="zb", bufs=2))
        xtp = ctx.enter_context(tc.tile_pool(name="xt", bufs=2))
        statp = ctx.enter_context(tc.tile_pool(name="stat", bufs=2))
        ep = ctx.enter_context(tc.tile_pool(name="eo", bufs=3))
        e16p = ctx.enter_context(tc.tile_pool(name="e16", bufs=2))
        simp = ctx.enter_context(tc.tile_pool(name="sim", bufs=2, space="PSUM"))

        ones_bf = singles.tile([128, 1], bf16)
        nc.vector.memset(ones_bf[:], 1.0)
        ln_half = singles.tile([128, 1], f32)
        nc.vector.memset(ln_half[:], float(np.log(0.5)))
        acc = singles.tile([128, 3 * L * RC + L * W], f32)
        nc.vector.memset(acc[:], 0.0)
        dsum = acc[:, 0 : L * RC]
        d16r = acc[:, L * RC : 2 * L * RC]
        pos = acc[:, 2 * L * RC : 3 * L * RC]
        cacc = acc[:, 3 * L * RC :].rearrange("p (l w) -> p l w", l=L)
        junk = singles.tile([128, D], f32)
        junkb = singles.tile([128, D], bf16)

        zbs, xts = {}, {}

        def prep(l):
            nat = natp.tile([128, W, D], f32)
            ssq = statp.tile([128, W], f32)
            lnssq = statp.tile([128, W], f32)
            invn = statp.tile([128, W], f32)
            zb = zbp.tile([128, W, D], bf16)
            xt = xtp.tile([128, W, 128], bf16)
            zbf = zb[:].rearrange("p s d -> p (s d)")
            for q0 in range(0, W, QS):
                q1 = q0 + QS
                nc.sync.dma_start(out=nat[:, q0:q1, :], in_=emb[:, l, q0:q1, :])
                for s in range(q0, q1):
                    nc.vector.scalar_tensor_tensor(
                        out=junk[:], in0=nat[:, s, :], scalar=1.0,
                        in1=nat[:, s, :], op0=OP.mult, op1=OP.mult,
                        accum_out=ssq[:, s : s + 1])
                nc.scalar.activation(out=lnssq[:, q0:q1], in_=ssq[:, q0:q1],
                                     func=AF.Ln)
                nc.scalar.activation(out=invn[:, q0:q1], in_=lnssq[:, q0:q1],
                                     func=AF.Exp, scale=-0.5)
                for s in range(q0, q1):
                    nc.gpsimd.tensor_scalar_mul(
                        zb[:, s, :], nat[:, s, :], invn[:, s : s + 1])
                nc.sync.dma_start_transpose(
                    out=xt[:, q0:q1, :], in_=zbf[:, q0 * D : q1 * D])
            # positives: distance-16 diagonal row-dots (needs only zb)
            for i in range(RC):
                nc.vector.scalar_tensor_tensor(
                    out=junkb[:], in0=zb[:, i, :], scalar=1.0,
                    in1=zb[:, i + SPAN, :], op0=OP.mult, op1=OP.mult,
                    accum_out=pos[:, l * RC + i : l * RC + i + 1])
            zbs[l], xts[l] = zb, xt

        def span_mm(l, i, st):
            xt = xts[l]
            xtf = xt[:].rearrange("p s r -> p (s r)")
            for q in range(4):
                c0 = i * 128 + q * 512
                nc.tensor.matmul(
                    st[:, q * 512 : (q + 1) * 512], xt[:, i, :],
                    xtf[:, c0 : c0 + 512], start=True, stop=True)

        def span_exp(l, i, st):
            e = ep.tile([128, SPAN * 128], bf16, tag="e")
            nc.scalar.activation(
                out=e[:], in_=st[:], func=AF.Exp, scale=INV_T,
                accum_out=dsum[:, l * RC + i : l * RC + i + 1])
            return e

        def span_colsum(l, i, st, e):
            for k in range(1, SPAN):
                nc.tensor.matmul(
                    st[:, k - 1 : k], e[:, k * 128 : (k + 1) * 128],
                    ones_bf[:], start=True, stop=True)
            nc.vector.tensor_tensor(
                out=cacc[:, l, i + 1 : i + SPAN],
                in0=cacc[:, l, i + 1 : i + SPAN],
                in1=st[:, 0 : SPAN - 1], op=OP.add)

        def sims(l):
            xt = xts[l]
            sts = [simp.tile([128, SPAN * 128], f32, tag="st", name=f"st{l}_{i}")
                   for i in range(RC)]
            es = {}
            # emission order keeps PE's in-order queue fed: each rc's
            # colsums (which wait on that rc's exp) are emitted after the
            # NEXT rc's span matmuls.
            span_mm(l, 0, sts[0])
            es[0] = span_exp(l, 0, sts[0])
            span_mm(l, 1, sts[1])
            es[1] = span_exp(l, 1, sts[1])
            span_colsum(l, 0, sts[0], es[0])
            span_mm(l, 2, sts[2])
            es[2] = span_exp(l, 2, sts[2])
            span_colsum(l, 1, sts[1], es[1])
            span_mm(l, 3, sts[3])
            es[3] = span_exp(l, 3, sts[3])
            span_colsum(l, 2, sts[2], es[2])
            # distance-16 blocks reuse st0's buffer (exp'd + colsummed)
            st2 = sts[0]
            for i in range(RC):
                nc.tensor.matmul(
                    st2[:, i * 128 : (i + 1) * 128], xt[:, i, :],
                    xt[:, i + SPAN, :], start=True, stop=True)
            e16 = e16p.tile([128, RC, 128], bf16)
            nc.scalar.activation(
                out=e16[:].rearrange("p a b -> p (a b)"), in_=st2[:, 0:512],
                func=AF.Exp, scale=INV_T, bias=ln_half[:])
            span_colsum(l, 3, sts[3], es[3])
            nc.vector.reduce_sum(
                out=d16r[:, l * RC : (l + 1) * RC], in_=e16[:], axis=AX.X)
            for i in range(RC):
                nc.tensor.matmul(
                    st2[:, 512 + i : 513 + i], e16[:, i, :], ones_bf[:],
                    start=True, stop=True)
            nc.vector.tensor_tensor(
                out=cacc[:, l, SPAN : SPAN + RC],
                in0=cacc[:, l, SPAN : SPAN + RC],
                in1=st2[:, 512 : 512 + RC], op=OP.add)

        prep(0)
        for l in range(L):
            if l + 1 < L:
                prep(l + 1)
            sims(l)

        nc.sync.dma_start(out=out[:, :], in_=acc[:])

    nc.finalize()
    _built = nc
    return nc


def _in_maps(emb_i, emb_j, joint_valid):
    emb_i = np.asarray(emb_i, dtype=np.float32)
    emb_j = np.asarray(emb_j, dtype=np.float32)
    jv = np.asarray(joint_valid, dtype=np.float32).reshape(-1)
    reps = np.concatenate(
        [emb_i.reshape(L, N, D), emb_j.reshape(L, N, D)], axis=1)  # [L, M, D]
    repsc = reps.reshape(L, NCH, 128, D)
    maps = []
    for c in range(NCORES):
        sel = (RC * c + np.arange(W)) % NCH
        win = repsc[:, sel]                       # [L, W, 128, D]
        nat = np.ascontiguousarray(win.transpose(2, 0, 1, 3))  # [128, L, W, D]
        maps.append({"emb_nat": nat})
    return maps, jv


def _combine(results, jv):
    E5 = float(np.exp(INV_T))  # self-similarity exp(1/T)
    denom = np.zeros((L, NCH, 128), dtype=np.float64)
    posg = np.zeros((L, NCH, 128), dtype=np.float64)
    nLR = L * RC
    for c, r in enumerate(results):
        a = r["out_all"].astype(np.float64)     # [128, 128]
        dsum = a[:, 0:nLR]
        d16r = a[:, nLR : 2 * nLR]
        pos = a[:, 2 * nLR : 3 * nLR]
        cacc = a[:, 3 * nLR :].reshape(128, L, W)
        for l in range(L):
            for i in range(RC):
                g = (RC * c + i) % NCH
                denom[l, g] += dsum[:, l * RC + i] + d16r[:, l * RC + i]
                posg[l, g] = pos[:, l * RC + i]
            for j in range(1, W):
                g = (RC * c + j) % NCH
                denom[l, g] += cacc[:, l, j]
    denom -= E5
    w = jv.astype(np.float64)                   # [N]
    wrow = np.concatenate([w, w]).reshape(NCH, 128)  # weight per global row
    lp = -posg * INV_T + np.log(denom)          # [L, NCH, 128]
    loss = (lp * wrow[None]).sum() / (2.0 * w.sum())
    return np.float32(loss)


def kernel(emb_i, emb_j, joint_valid):
    from concourse.bass_utils import run_bass_kernel_spmd

    nc = _build()
    maps, jv = _in_maps(emb_i, emb_j, joint_valid)
    res = run_bass_kernel_spmd(nc, maps, core_ids=list(range(NCORES)))
    return _combine(res.results, jv)


def run_traced(inputs, trace_cores=None):
    """test.py helper: same run but with NTFF tracing enabled."""
    from concourse.bass_utils import run_bass_kernel_spmd

    nc = _build()
    maps, jv = _in_maps(**inputs)
    res = run_bass_kernel_spmd(
        nc, maps, core_ids=list(range(NCORES)), trace=True,
        trace_cores=trace_cores if trace_cores is not None else list(range(NCORES)))
    res.loss = _combine(res.results, jv)
    return res


# revision 13
# speedup vs baseline: 1.2699x; 1.2699x over previous
"""Distributed Trainium2 (Bass/Tile) kernel for the contrastive loss.

Ring-partitioned symmetric-similarity scheme (8 NeuronCores, SPMD):

  Global per l: 4096 rows in 32 chunks of 128.  sim = Z Z^T is symmetric,
  so each unordered chunk pair is computed ONCE: chunk i covers column
  chunks {i..i+15} (ring-forward), and the distance-16 block is computed
  by both endpoints with its exp HALVED (bias ln 1/2).  Core c owns row
  chunks {4c..4c+3}; it therefore only needs Z chunks {4c..4c+19} (a
  20-chunk window, rolled so the window is local chunks 0..19).

  Per core:
    - load raw window rows in natural layout [128, l, 20, 128] (fp32),
    - ssq via fused square+row-sum on DVE; invn = exp(-0.5 ln ssq) (ACT),
    - scale rows by invn -> bf16 zb (GpSimd tensor_scalar),
    - transpose zb via the XBAR DMA-transpose -> xt [d, chunk, row] bf16,
      (prep chain runs in quarters so the first matmul starts early),
    - row-chunk i: 4x 512-col bf16 matmuls -> PSUM [128, 2048];
      ACT exp (scale 1/T) -> E bf16 SBUF + per-row accum (denominator
      row part);  per covered column chunk a 1-col ones-matmul on PE
      computes the column sums (the denominator part owed to OTHER
      row chunks); distance-16 block separately with bias ln(1/2),
    - positive pairs are exactly the distance-16 diagonals:
      pos = zb[:,i,:] . zb[:,i+16,:] row-dots on DVE.
  Outputs per core (one packed DMA): row accums, d16 row sums, pos dots
  and column-sum partials.  The host does the tiny cross-core assembly:
  denominators = row part + mapped column partials - e^5 (self term),
  then loss = sum w (-pos/T + log denom) / (2 sum w).
"""

import numpy as np

TEMP = 0.2
INV_T = 1.0 / TEMP
L, B, K, D = 4, 64, 32, 128
N = B * K            # 2048
M = 2 * N            # 4096 rows per l
NCH = 32             # global 128-row chunks per l
NCORES = 8
RC = 4               # row chunks owned per core
W = 20               # chunk window per core (rc spans + d16 partners)
SPAN = 16            # forward span chunks (excl. the halved d16 block)
QS = 5               # prep quarter size (chunks per quarter)

_built = None


def _build():
    global _built
    if _built is not None:
        return _built
    from contextlib import ExitStack

    import concourse.tile as tile
    from concourse import bacc
    import concourse.mybir as mybir

    f32 = mybir.dt.float32
    bf16 = mybir.dt.bfloat16
    AF = mybir.ActivationFunctionType
    OP = mybir.AluOpType
    AX = mybir.AxisListType

    # Pin every ACT op to the natural_log_exp_and_others table set (covers
    # Copy/Exp/Identity/Ln) so bacc emits exactly one LoadActFuncSet.
    from concourse import hw_specs as _hw
    _tabs = dict(_hw.get_activation_tables("gen3"))
    _pinned = {
        name: (fns if name == "natural_log_exp_and_others" else frozenset())
        for name, fns in _tabs.items()
    }
    _hw.get_activation_tables.cache_clear()
    _orig = _hw.get_activation_tables.__wrapped__

    def _patched(arch):
        if arch == "gen3":
            return _pinned
        return _orig(arch)

    _hw.get_activation_tables = _patched
    import concourse.bacc as _baccmod
    if hasattr(_baccmod, "get_activation_tables"):
        _baccmod.get_activation_tables = _patched

    nc = bacc.Bacc(None, target_bir_lowering=False)
    emb = nc.dram_tensor("emb_nat", [128, L, W, D], f32, kind="ExternalInput")
    # packed: [dsum(16) | d16r(16) | pos(16) | cacc(L*W=80)] = 128 cols
    out = nc.dram_tensor("out_all", [128, 3 * L * RC + L * W], f32,
                         kind="ExternalOutput")

    with ExitStack() as ctx:
        tc = ctx.enter_context(tile.TileContext(nc))
        singles = ctx.enter_context(tc.tile_pool(name="singles", bufs=1))
        natp = ctx.enter_context(tc.tile_pool(name="nat", bufs=2))
        zbp = ctx.enter_context(tc.tile_pool(name="zb", bufs=2))
        xtp = ctx.enter_context(tc.tile_pool(name="xt", bufs=2))
        statp = ctx.enter_context(tc.tile_pool(name="stat", bufs=2))
        ep = ctx.enter_context(tc.tile_pool(name="eo", bufs=3))
        e16p = ctx.enter_context(tc.tile_pool(name="e16", bufs=2))
        simp = ctx.enter_context(tc.tile_pool(name="sim", bufs=2, space="PSUM"))

        ones_bf = singles.tile([128, 1], bf16)
        nc.vector.memset(ones_bf[:], 1.0)
        ln_half = singles.tile([128, 1], f32)
        nc.vector.memset(ln_half[:], float(np.log(0.5)))
        acc = singles.tile([128, 3 * L * RC + L * W], f32)
        nc.vector.memset(acc[:], 0.0)
        dsum = acc[:, 0 : L * RC]
        d16r = acc[:, L * RC : 2 * L * RC]
        pos = acc[:, 2 * L * RC : 3 * L * RC]
        cacc = acc[:, 3 * L * RC :].rearrange("p (l w) -> p l w", l=L)
        junk = singles.tile([128, D], f32)
        junkb = singles.tile([128, D], bf16)

        zbs, xts = {}, {}

        def prep(l):
            nat = natp.tile([128, W, D], f32)
            ssq = statp.tile([128, W], f32)
            lnssq = statp.tile([128, W], f32)
            invn = statp.tile([128, W], f32)
            zb = zbp.tile([128, W, D], bf16)
            xt = xtp.tile([128, W, 128], bf16)
            zbf = zb[:].rearrange("p s d -> p (s d)")
            qs = QS if l == 0 else W
            # all input DMAs first: a waiting XBAR on SP.SEQ must not
            # delay later load quarters
            for q0 in range(0, W, qs):
                nc.sync.dma_start(
                    out=nat[:, q0 : q0 + qs, :], in_=emb[:, l, q0 : q0 + qs, :])
            for q0 in range(0, W, qs):
                q1 = q0 + qs
                for s in range(q0, q1):
                    nc.vector.scalar_tensor_tensor(
                        out=junk[:], in0=nat[:, s, :], scalar=1.0,
                        in1=nat[:, s, :], op0=OP.mult, op1=OP.mult,
                        accum_out=ssq[:, s : s + 1])
                nc.scalar.activation(out=lnssq[:, q0:q1], in_=ssq[:, q0:q1],
                                     func=AF.Ln)
                nc.scalar.activation(out=invn[:, q0:q1], in_=lnssq[:, q0:q1],
                                     func=AF.Exp, scale=-0.5)
                for s in range(q0, q1):
                    nc.gpsimd.tensor_scalar_mul(
                        zb[:, s, :], nat[:, s, :], invn[:, s : s + 1])
                nc.sync.dma_start_transpose(
                    out=xt[:, q0:q1, :], in_=zbf[:, q0 * D : q1 * D])
            zbs[l], xts[l] = zb, xt

        def span_mm(l, i, st):
            xt = xts[l]
            xtf = xt[:].rearrange("p s r -> p (s r)")
            for q in range(4):
                c0 = i * 128 + q * 512
                nc.tensor.matmul(
                    st[:, q * 512 : (q + 1) * 512], xt[:, i, :],
                    xtf[:, c0 : c0 + 512], start=True, stop=True)

        def span_exp(l, i, st):
            e = ep.tile([128, SPAN * 128], bf16, tag="e")
            nc.scalar.activation(
                out=e[:], in_=st[:], func=AF.Exp, scale=INV_T,
                accum_out=dsum[:, l * RC + i : l * RC + i + 1])
            return e

        def span_colsum(l, i, st, e):
            for k in range(1, SPAN):
                nc.tensor.matmul(
                    st[:, k - 1 : k], e[:, k * 128 : (k + 1) * 128],
                    ones_bf[:], start=True, stop=True)
            nc.vector.tensor_tensor(
                out=cacc[:, l, i + 1 : i + SPAN],
                in0=cacc[:, l, i + 1 : i + SPAN],
                in1=st[:, 0 : SPAN - 1], op=OP.add)

        def sims(l):
            xt = xts[l]
            zb = zbs[l]
            # distance-16 blocks first: tiny matmuls + exp fill the ACT
            # bubble at the l-transition while the span buffers drain
            st2 = simp.tile([128, SPAN * 128], f32, tag="st", name=f"d16st{l}")
            for i in range(RC):
                nc.tensor.matmul(
                    st2[:, i * 128 : (i + 1) * 128], xt[:, i, :],
                    xt[:, i + SPAN, :], start=True, stop=True)
            sts = [simp.tile([128, SPAN * 128], f32, tag="st", name=f"st{l}_{i}")
                   for i in range(RC)]
            es = {}
            span_mm(l, 0, sts[0])
            e16 = e16p.tile([128, RC, 128], bf16)
            nc.scalar.activation(
                out=e16[:].rearrange("p a b -> p (a b)"), in_=st2[:, 0:512],
                func=AF.Exp, scale=INV_T, bias=ln_half[:])
            es[0] = span_exp(l, 0, sts[0])
            span_mm(l, 1, sts[1])
            for i in range(RC):
                nc.tensor.matmul(
                    st2[:, 512 + i : 513 + i], e16[:, i, :], ones_bf[:],
                    start=True, stop=True)
            nc.vector.reduce_sum(
                out=d16r[:, l * RC : (l + 1) * RC], in_=e16[:], axis=AX.X)
            nc.vector.tensor_tensor(
                out=cacc[:, l, SPAN : SPAN + RC],
                in0=cacc[:, l, SPAN : SPAN + RC],
                in1=st2[:, 512 : 512 + RC], op=OP.add)
            es[1] = span_exp(l, 1, sts[1])
            span_mm(l, 2, sts[2])
            span_colsum(l, 0, sts[0], es[0])
            es[2] = span_exp(l, 2, sts[2])
            span_mm(l, 3, sts[3])
            span_colsum(l, 1, sts[1], es[1])
            es[3] = span_exp(l, 3, sts[3])
            span_colsum(l, 2, sts[2], es[2])
            span_colsum(l, 3, sts[3], es[3])
            # positives: distance-16 diagonal row-dots (needs only zb);
            # emitted last so DVE's in-order queue isn't blocked on them
            for i in range(RC):
                nc.vector.scalar_tensor_tensor(
                    out=junkb[:], in0=zb[:, i, :], scalar=1.0,
                    in1=zb[:, i + SPAN, :], op0=OP.mult, op1=OP.mult,
                    accum_out=pos[:, l * RC + i : l * RC + i + 1])

        prep(0)
        for l in range(L):
            if l + 1 < L:
                prep(l + 1)
            sims(l)

        nc.sync.dma_start(out=out[:, :], in_=acc[:])

    nc.finalize()
    _built = nc
    return nc


def _in_maps(emb_i, emb_j, joint_valid):
    emb_i = np.asarray(emb_i, dtype=np.float32)
    emb_j = np.asarray(emb_j, dtype=np.float32)
    jv = np.asarray(joint_valid, dtype=np.float32).reshape(-1)
    reps = np.concatenate(
        [emb_i.reshape(L, N, D), emb_j.reshape(L, N, D)], axis=1)  # [L, M, D]
    repsc = reps.reshape(L, NCH, 128, D)
    maps = []
    for c in range(NCORES):
        sel = (RC * c + np.arange(W)) % NCH
        win = repsc[:, sel]                       # [L, W, 128, D]
        nat = np.ascontiguousarray(win.transpose(2, 0, 1, 3))  # [128, L, W, D]
        maps.append({"emb_nat": nat})
    return maps, jv


def _combine(results, jv):
    E5 = float(np.exp(INV_T))  # self-similarity exp(1/T)
    denom = np.zeros((L, NCH, 128), dtype=np.float64)
    posg = np.zeros((L, NCH, 128), dtype=np.float64)
    nLR = L * RC
    for c, r in enumerate(results):
        a = r["out_all"].astype(np.float64)     # [128, 128]
        dsum = a[:, 0:nLR]
        d16r = a[:, nLR : 2 * nLR]
        pos = a[:, 2 * nLR : 3 * nLR]
        cacc = a[:, 3 * nLR :].reshape(128, L, W)
        for l in range(L):
            for i in range(RC):
                g = (RC * c + i) % NCH
                denom[l, g] += dsum[:, l * RC + i] + d16r[:, l * RC + i]
                posg[l, g] = pos[:, l * RC + i]
            for j in range(1, W):
                g = (RC * c + j) % NCH
                denom[l, g] += cacc[:, l, j]
    denom -= E5
    w = jv.astype(np.float64)                   # [N]
    wrow = np.concatenate([w, w]).reshape(NCH, 128)  # weight per global row
    lp = -posg * INV_T + np.log(denom)          # [L, NCH, 128]
    loss = (lp * wrow[None]).sum() / (2.0 * w.sum())
    return np.float32(loss)


def kernel(emb_i, emb_j, joint_valid):
    from concourse.bass_utils import run_bass_kernel_spmd

    nc = _build()
    maps, jv = _in_maps(emb_i, emb_j, joint_valid)
    res = run_bass_kernel_spmd(nc, maps, core_ids=list(range(NCORES)))
    return _combine(res.results, jv)


def run_traced(inputs, trace_cores=None):
    """test.py helper: same run but with NTFF tracing enabled."""
    from concourse.bass_utils import run_bass_kernel_spmd

    nc = _build()
    maps, jv = _in_maps(**inputs)
    res = run_bass_kernel_spmd(
        nc, maps, core_ids=list(range(NCORES)), trace=True,
        trace_cores=trace_cores if trace_cores is not None else list(range(NCORES)))
    res.loss = _combine(res.results, jv)
    return res
